# revision 2
# baseline (speedup 1.0000x reference)
"""Trainium2 Bass kernel for IrrepWiseLinear.

out[n, m, :] = x[n, m, :] @ weight[seg_id(m)]   (seg sizes [1,3,5,7], DIM=16)

Strategy: data-parallel over the 8 NeuronCores on the leading N dim.
Per core: stream x in big blocks of T*128 nodes ([128, T, 16, 128] f32,
contiguous 8KB runs per partition line), PE-transpose each per-m
[128n, 128c] slice (fp32 transpose mode), copy PSUM->SBUF (DVE), then fp32
matmul with the per-path weight (lhsT = x_m^T [c, n], rhs = W[path] [c, d])
giving out [n, d] in natural order (copied out PSUM->SBUF on ACT);
reassemble [128, T*2048] blocks and store with big DMAs.
"""

import sys

sys.path.insert(0, "/opt/trn_rl_repo")

import numpy as np

# hardcoded problem shape (self-contained; do not read spec/reference)
N = 65536
DIM = 16
C_IN = 128
C_OUT = 128
NUM_PATHS = 4
SEG_IDS = [0, 1, 1, 1, 2, 2, 2, 2, 2, 3, 3, 3, 3, 3, 3, 3]
N_CORES = 8
N_SHARD = N // N_CORES  # 8192 nodes per core

# tunables
CONFIG = {
    "sub_blocks": 2,      # T: 128-node sub-blocks per DMA block (T*1MB DMAs)
    "in_bufs": 3,
    "out_bufs": 3,
    "xt_bufs": 4,
    "psum_bufs": 3,
    "m_group": 4,         # m's per PSUM bank group
    "xt_dtype": "float32",   # or "float32r" for the transpose pass
}

_cache = {}


def _build():
    import concourse.bass as bass
    import concourse.mybir as mybir
    import concourse.tile as tile
    from concourse import bacc
    from concourse.masks import make_identity

    f32 = mybir.dt.float32
    cfg = dict(CONFIG)
    T = cfg["sub_blocks"]
    MG = cfg["m_group"]
    BLOCK = 128 * T
    n_blocks = N_SHARD // BLOCK
    assert N_SHARD % BLOCK == 0 and DIM % MG == 0

    nc = bacc.Bacc("TRN2", target_bir_lowering=False, debug=False,
                   num_devices=N_CORES)
    x_d = nc.dram_tensor("x", [N_SHARD, DIM, C_IN], f32, kind="ExternalInput")
    w_d = nc.dram_tensor("w", [NUM_PATHS, C_IN, C_OUT], f32,
                         kind="ExternalInput")
    o_d = nc.dram_tensor("out", [N_SHARD, DIM, C_OUT], f32,
                         kind="ExternalOutput")

    x_ap = x_d.ap().rearrange("(b t p) m c -> b p t m c", p=128, t=T)
    o_ap = o_d.ap().rearrange("(b t p) m d -> b p t m d", p=128, t=T)

    with tile.TileContext(nc) as tc:
        with (
            tc.tile_pool(name="const", bufs=1) as const_pool,
            tc.tile_pool(name="xin", bufs=cfg["in_bufs"]) as in_pool,
            tc.tile_pool(name="xout", bufs=cfg["out_bufs"]) as out_pool,
            tc.tile_pool(name="xt_sb", bufs=cfg["xt_bufs"]) as xts_pool,
            tc.tile_pool(name="xt_ps", bufs=cfg["psum_bufs"],
                         space="PSUM") as xtp_pool,
            tc.tile_pool(name="o_ps", bufs=cfg["psum_bufs"],
                         space="PSUM") as outp_pool,
        ):
            ident = const_pool.tile([128, 128], f32)
            make_identity(nc, ident[:])

            # weight in SBUF: [c, path, d]
            w_sb = const_pool.tile([C_IN, NUM_PATHS, C_OUT], f32)
            nc.sync.dma_start(w_sb[:], w_d.ap().rearrange("p c d -> c p d"))

            for b in range(n_blocks):
                in_t = in_pool.tile([128, T, DIM, C_IN], f32)
                nc.sync.dma_start(in_t[:], x_ap[b])
                out_t = out_pool.tile([128, T, DIM, C_OUT], f32)

                for t in range(T):
                    for g in range(DIM // MG):
                        xt_ps = xtp_pool.tile([C_IN, MG * 128], f32)
                        for j in range(MG):
                            m = g * MG + j
                            nc.tensor.transpose(
                                xt_ps[:, j * 128:(j + 1) * 128],
                                in_t[:, t, m, :],
                                ident[:],
                            )
                        xt_sb = xts_pool.tile([C_IN, MG * 128], f32)
                        nc.vector.tensor_copy(xt_sb[:], xt_ps[:])

                        o_ps = outp_pool.tile([128, MG * C_OUT], f32)
                        for j in range(MG):
                            m = g * MG + j
                            nc.tensor.matmul(
                                o_ps[:, j * C_OUT:(j + 1) * C_OUT],
                                lhsT=xt_sb[:, j * 128:(j + 1) * 128],
                                rhs=w_sb[:, SEG_IDS[m], :],
                                start=True, stop=True,
                            )
                        nc.scalar.copy(
                            out=out_t[:, t, g * MG:(g + 1) * MG, :],
                            in_=o_ps[:],
                        )

                nc.scalar.dma_start(o_ap[b], out_t[:])

    nc.compile()
    return nc


def _get_nc():
    if "nc" not in _cache:
        _cache["nc"] = _build()
    return _cache["nc"]


def _run(x, weight, trace=False, **trace_kw):
    from concourse.bass_utils import run_bass_kernel_spmd

    nc = _get_nc()
    x = np.ascontiguousarray(x, dtype=np.float32)
    weight = np.ascontiguousarray(weight, dtype=np.float32)
    in_maps = [
        {"x": x[i * N_SHARD:(i + 1) * N_SHARD], "w": weight}
        for i in range(N_CORES)
    ]
    res = run_bass_kernel_spmd(nc, in_maps, list(range(N_CORES)),
                               trace=trace, **trace_kw)
    out = np.concatenate([res.results[i]["out"] for i in range(N_CORES)],
                         axis=0)
    return out, res


def kernel(x, weight):
    out, _ = _run(x, weight, trace=False)
    return out


if __name__ == "__main__":
    rng = np.random.default_rng(0)
    x = rng.standard_normal((N, DIM, C_IN), dtype=np.float32)
    w = rng.standard_normal((NUM_PATHS, C_IN, C_OUT), dtype=np.float32)
    w /= np.sqrt(C_IN)
    out = kernel(x, w)
    w_rows = w[SEG_IDS]
    exp = np.einsum("nmc,mcd->nmd", x, w_rows)
    err = np.abs(out - exp).max() / np.abs(exp).max()
    print("rel err:", err)
